# revision 4
# baseline (speedup 1.0000x reference)
"""Int8 per-token-quantized linear (MluQuantLinearInt8) on 8 Trainium2 cores.

  out[s, n] = (sum_k q[s,k] * w[n,k]) * x_scale[s] * w_scale[n]
  q = round(x / x_scale) clipped to [-127, 127],  x_scale = max(|x|_row, 1e-8)/127

v2: quantization moved to the HOST (bit-exact vs the reference, numpy f32);
the device program is a pure weights-streaming GEMM + fused dequant.
Sharding: data-parallel over tokens (512/core); weights replicated.
Host pre-packs: qT bf16 [KC, P, S_C] (kc-sliced so the GEMM starts while
qT still streams), weights bf16 [WC, NSUB, P, KC, P] (int8 exact in bf16;
each (wc,sub) slice is one contiguous 1 MB DMA), xsb = x_scale broadcast
[P, S_C], ws packed [P, NT]. Per-core GEMM is weights-stationary:
lhsT = wtile[128k, 128n], rhs = qT[128k, 512tok], psum [128n, 512tok];
dequant fused into the PSUM->SBUF eviction; output stored transposed
[N, S_C] and re-assembled on host.
"""

import sys
from contextlib import ExitStack
from functools import lru_cache

import numpy as np

for _p in ("/opt/trn_rl_repo", "/root/.axon_site/_ro/trn_rl_repo"):
    if _p not in sys.path:
        sys.path.append(_p)

import ml_dtypes  # noqa: E402

import concourse.bass as bass  # noqa: E402
import concourse.bass2jax as bass2jax  # noqa: E402
import concourse.mybir as mybir  # noqa: E402
import concourse.tile as tile  # noqa: E402
from concourse.bass_utils import (  # noqa: E402
    compile_bir_kernel as _orig_compile_bir_kernel,
    run_bass_kernel_spmd,
)

# The walrus build in this container accepts only ONE sync-wait per
# instruction ("Too many sync wait commands", CoreV3GenImpl setupSyncWait) —
# Tile's kernel-tail drain carries several. Split extra waits onto preceding
# single-wait EventSemaphore carriers on the same engine (engine program order
# makes the AND of waits equivalent).
import json as _json  # noqa: E402


def _split_multi_waits(bir_json):
    d = _json.loads(bir_json)
    changed = False
    for fn in d.get("functions", []):
        for bb in fn.get("blocks", []) or []:
            insts = bb.get("instructions")
            if not insts:
                continue
            out = []
            for ins in insts:
                si = ins.get("sync_info")
                waits = (si or {}).get("on_wait") or []
                if len(waits) > 1:
                    for j, w in enumerate(waits[:-1]):
                        out.append(
                            {
                                "engine": ins.get("engine"),
                                "ins": [],
                                "outs": [],
                                "name": f"{ins.get('name', 'I')}_w{j}",
                                "opcode": "EventSemaphore",
                                "sync_info": {"on_update": [], "on_wait": [w]},
                            }
                        )
                    si["on_wait"] = [waits[-1]]
                    changed = True
                out.append(ins)
            bb["instructions"] = out
    if not changed:
        return bir_json
    return _json.dumps(d).encode()


def _patched_compile_bir_kernel(bir_json, tmpdir, neff_name="file.neff"):
    return _orig_compile_bir_kernel(
        _split_multi_waits(bir_json), tmpdir, neff_name=neff_name
    )


bass2jax.compile_bir_kernel = _patched_compile_bir_kernel

P = 128
NCORES = 8
S, K_FULL, N_FULL = 4096, 4096, 16384
QMAX = 127.0
F32 = mybir.dt.float32
BF16 = mybir.dt.bfloat16
INT8 = mybir.dt.int8


def build_nc(S_C, K, N, NSUB=4, WK=8):
    """One-core program; SPMD-replicated across cores by the runner.

    Inputs (per core):
      qT  [KC, P, S_C] bf16 - host-quantized activations, transposed
      xsb [P, S_C] f32 - per-token scale, broadcast across partitions
      wt  [WC, NSUB, P, KC, P] bf16 - weights, host-packed SBUF-layout
      ws  [P, NT]   f32 - weight_scale packed ws[p, nt] = weight_scale[nt*128+p]
    Output:
      outT [N, S_C] f32 - dequantized output, transposed

    DMA plan: qT + first weight chunk are kc-chunked (WK blocks per piece)
    and interleaved so the first psum chain starts ~2 MB into the stream;
    the remaining weight chunks stream as whole (wc, sub) 1 MB slices on
    the gpsimd queue while qT/out traffic rides the sync queue.
    """
    KC = K // P  # contraction blocks
    NT = N // P  # output-channel tiles (one psum chain each)
    WC = NT // NSUB  # streamed weight chunks

    nc = bass.Bass()
    q8 = nc.declare_dram_parameter("q8", [P, KC, S_C], INT8, isOutput=False)
    xsb_d = nc.declare_dram_parameter("xsb", [P, S_C], F32, isOutput=False)
    wt = nc.declare_dram_parameter("wt", [WC, NSUB, P, KC, P], BF16, isOutput=False)
    ws = nc.declare_dram_parameter("ws", [P, NT], F32, isOutput=False)
    outT = nc.declare_dram_parameter("outT", [N, S_C], F32, isOutput=True)

    outT_t = outT.rearrange("(nt p) s -> nt p s", p=P)

    with tile.TileContext(nc) as tc, ExitStack() as ctx:
        const_pool = ctx.enter_context(tc.tile_pool(name="const", bufs=1))
        qpool = ctx.enter_context(tc.tile_pool(name="qp", bufs=1))
        wpool = ctx.enter_context(tc.tile_pool(name="wp", bufs=3))
        opool = ctx.enter_context(tc.tile_pool(name="op", bufs=4))
        ps_pool = ctx.enter_context(tc.tile_pool(name="psp", bufs=8, space="PSUM"))

        q8_sb = qpool.tile([P, KC, S_C], INT8)
        qT_sb = qpool.tile([P, KC, S_C], BF16)
        xsb = const_pool.tile([P, S_C], F32)
        ws_sb = const_pool.tile([P, NT], F32)

        # ---- PE pre-warm: junk matmuls on memset tiles ride out the DMA
        # wait and the tensor engine's cold-clock p-state ramp.
        junk_w = const_pool.tile([P, P], BF16)
        junk_r = const_pool.tile([P, S_C], BF16)
        nc.vector.memset(junk_w, 0.0)
        nc.vector.memset(junk_r, 0.0)
        junk_ps = ps_pool.tile([P, S_C], F32, name="ps", tag="ps")
        for i in range(14):
            nc.tensor.matmul(
                junk_ps, lhsT=junk_w, rhs=junk_r, start=(i == 0), stop=(i == 13)
            )
        junk_out = const_pool.tile([P, 1], F32)
        nc.vector.tensor_copy(junk_out, junk_ps[:, :1])

        # ---- DMA issue order (sync queue): q8 (kc-chunked) interleaved
        # with chunk-0 weights, then the small consts. Each q8 block is
        # upconverted int8->bf16 split across DVE/Act/Pool so conversion
        # keeps ahead of the first psum chain's consumption.
        def load_q(kc0, kcn):
            nc.sync.dma_start(
                q8_sb[:, kc0 : kc0 + kcn, :], q8[:, kc0 : kc0 + kcn, :]
            )

        def convert_q(kc0):
            # NOTE: keep this coarse (3 ops/block). Per-kc slicing measured
            # ~6 us WORSE on a fast-clock run: the fixed per-op engine
            # overhead of 32 tiny copies outweighs the conversion-latency
            # it saves, adding ~6 us of early chain stalls.
            nc.vector.tensor_copy(
                qT_sb[:, kc0 : kc0 + 3, :], q8_sb[:, kc0 : kc0 + 3, :]
            )
            nc.scalar.copy(
                qT_sb[:, kc0 + 3 : kc0 + 6, :], q8_sb[:, kc0 + 3 : kc0 + 6, :]
            )
            nc.gpsimd.tensor_copy(
                qT_sb[:, kc0 + 6 : kc0 + 8, :], q8_sb[:, kc0 + 6 : kc0 + 8, :]
            )

        def load_w_sub(wtile, wc, sub, engine, kc0=0, kcn=None):
            kcn = kcn if kcn is not None else KC
            engine.dma_start(
                wtile[:, sub, kc0 : kc0 + kcn, :], wt[wc, sub, :, kc0 : kc0 + kcn, :]
            )

        def load_wchunk(wc):
            wtile = wpool.tile([P, NSUB, KC, P], BF16, tag="wtile")
            for sub in range(NSUB):
                load_w_sub(wtile, wc, sub, nc.sync)
            return wtile

        wtiles = {}
        wtiles[0] = wpool.tile([P, NSUB, KC, P], BF16, name="wtile", tag="wtile")
        # Interleave the q8 stream with chunk-0 weights so chain 0 starts
        # ~1 MB into the stream and then drips along the q8 arrivals while
        # later chains' weights trickle in behind: per 8-block of kc, ship
        # q8[kc-block] then w0s0[kc-block] then w0s1[kc-block]; w0s2/s3 after.
        for kc0 in range(0, KC, 8):
            load_q(kc0, 8)
            load_w_sub(wtiles[0], 0, 0, nc.sync, kc0, 8)
            load_w_sub(wtiles[0], 0, 1, nc.sync, kc0, 8)
            convert_q(kc0)
        for sub in range(2, NSUB):
            load_w_sub(wtiles[0], 0, sub, nc.sync)
        nc.sync.dma_start(xsb, xsb_d[:, :])
        nc.sync.dma_start(ws_sb, ws[:, :])
        if WC > 1:
            wtiles[1] = load_wchunk(1)
        if WC > 2:
            wtiles[2] = load_wchunk(2)

        # ---- weights-stationary GEMM + fused dequant ----
        # 2-chunk-deep prefetch: issue wc+2's loads at iteration wc's top so
        # the transfer overlaps two full chunk-compute windows.
        for wc in range(WC):
            if wc + 2 < WC and (wc + 2) not in wtiles:
                wtiles[wc + 2] = load_wchunk(wc + 2)
            wtile = wtiles.pop(wc)
            if wc == 0:
                # Chains 0 and 1 interleaved kc-block-wise: one chain alone
                # consumes q8+w0 faster than the ~310 GB/s fabric delivers,
                # so alternating two chains over each delivered block turns
                # delivery-bound PE idle into back-to-back matmuls.
                pss = [
                    ps_pool.tile([P, S_C], F32, name="ps", tag="ps")
                    for _ in range(2)
                ]
                for kcb in range(0, KC, 8):
                    for sub in (0, 1):
                        for kc in range(kcb, kcb + 8):
                            nc.tensor.matmul(
                                pss[sub],
                                lhsT=wtile[:, sub, kc, :],
                                rhs=qT_sb[:, kc, :],
                                start=(kc == 0),
                                stop=(kc == KC - 1),
                            )
                sub_plan = [(0, pss[0]), (1, pss[1])] + [
                    (s, None) for s in range(2, NSUB)
                ]
            else:
                sub_plan = [(s, None) for s in range(NSUB)]
            for sub, ps in sub_plan:
                nt = wc * NSUB + sub
                if ps is None:
                    ps = ps_pool.tile([P, S_C], F32, name="ps", tag="ps")
                    for kc in range(KC):
                        nc.tensor.matmul(
                            ps,
                            lhsT=wtile[:, sub, kc, :],
                            rhs=qT_sb[:, kc, :],
                            start=(kc == 0),
                            stop=(kc == KC - 1),
                        )
                out_sb = opool.tile([P, S_C], F32)
                # out = (acc * w_scale[n]) * x_scale[tok]
                nc.vector.scalar_tensor_tensor(
                    out=out_sb,
                    in0=ps,
                    scalar=ws_sb[:, nt : nt + 1],
                    in1=xsb,
                    op0=mybir.AluOpType.mult,
                    op1=mybir.AluOpType.mult,
                )
                nc.sync.dma_start(outT_t[nt], out_sb)

    return nc


def pack_inputs(input_tensor, weight, weight_scale, S_C, K, N, NSUB=4):
    """Host-side prep: quantize (bit-exact vs reference), transpose/shard q,
    pack weights to bf16 SBUF-chunk layout."""
    KC = K // P
    NT = N // P
    WC = NT // NSUB
    ncores = (input_tensor.size // K) // S_C

    x = np.ascontiguousarray(input_tensor.reshape(-1, K), dtype=np.float32)
    # per-token dynamic absmax int8 quantization (mirrors the f32 reference)
    amax = np.max(np.abs(x), axis=1, keepdims=True)
    x_scale = (np.maximum(amax, np.float32(1e-8)) / np.float32(QMAX)).astype(
        np.float32
    )
    q = np.clip(np.rint(x / x_scale), -QMAX, QMAX)
    # q8[c, p, kc, s] = q[c*S_C + s, kc*P + p]
    q8 = np.ascontiguousarray(
        q.astype(np.int8)
        .reshape(ncores, S_C, KC, P)
        .transpose(0, 3, 2, 1)
    )
    xsb = np.ascontiguousarray(
        np.broadcast_to(
            x_scale.reshape(ncores, 1, S_C), (ncores, P, S_C)
        )
    ).astype(np.float32)

    w_bf = weight.astype(ml_dtypes.bfloat16)  # [N, K], int8 values exact
    # wt[wc, sub, p, kc, i] = w[(wc*NSUB + sub)*P + i, kc*P + p]
    wt = np.ascontiguousarray(
        w_bf.reshape(WC, NSUB, P, KC, P).transpose(0, 1, 4, 3, 2)
    )
    ws = np.ascontiguousarray(
        weight_scale.reshape(NT, P).T.astype(np.float32)
    )  # [P, NT]
    return q8, xsb, wt, ws


@lru_cache(maxsize=2)
def _compiled_nc(S_C, K, N, NSUB, WK):
    return build_nc(S_C, K, N, NSUB=NSUB, WK=WK)


def run(input_tensor, weight, weight_scale, n_cores=NCORES, trace=False,
        NSUB=4, WK=8, **_compat_ignored):
    Sfull, K = input_tensor.shape[-2], input_tensor.shape[-1]
    N = weight.shape[0]
    S_C = Sfull // n_cores
    q8, xsb, wt, ws = pack_inputs(
        input_tensor, weight, weight_scale, S_C, K, N, NSUB
    )
    nc = _compiled_nc(S_C, K, N, NSUB, WK)
    in_maps = [
        {"q8": q8[c], "xsb": xsb[c], "wt": wt, "ws": ws}
        for c in range(n_cores)
    ]
    res = run_bass_kernel_spmd(nc, in_maps, core_ids=list(range(n_cores)), trace=trace)
    out = np.empty((Sfull, N), np.float32)
    for c in range(n_cores):
        out[c * S_C : (c + 1) * S_C] = res.results[c]["outT"].T
    return out[None], res


def kernel(input_tensor, weight, weight_scale):
    out, _ = run(
        np.asarray(input_tensor), np.asarray(weight), np.asarray(weight_scale)
    )
    return out


# revision 5
# speedup vs baseline: 1.2000x; 1.2000x over previous
"""Int8 per-token-quantized linear (MluQuantLinearInt8) on 8 Trainium2 cores.

  out[s, n] = (sum_k q[s,k] * w[n,k]) * x_scale[s] * w_scale[n]
  q = round(x / x_scale) clipped to [-127, 127],  x_scale = max(|x|_row, 1e-8)/127

v2: quantization moved to the HOST (bit-exact vs the reference, numpy f32);
the device program is a pure weights-streaming GEMM + fused dequant.
Sharding: data-parallel over tokens (512/core); weights replicated.
Host pre-packs: qT bf16 [KC, P, S_C] (kc-sliced so the GEMM starts while
qT still streams), weights bf16 [WC, NSUB, P, KC, P] (int8 exact in bf16;
each (wc,sub) slice is one contiguous 1 MB DMA), xsb = x_scale broadcast
[P, S_C], ws packed [P, NT]. Per-core GEMM is weights-stationary:
lhsT = wtile[128k, 128n], rhs = qT[128k, 512tok], psum [128n, 512tok];
dequant fused into the PSUM->SBUF eviction; output stored transposed
[N, S_C] and re-assembled on host.
"""

import sys
from contextlib import ExitStack
from functools import lru_cache

import numpy as np

for _p in ("/opt/trn_rl_repo", "/root/.axon_site/_ro/trn_rl_repo"):
    if _p not in sys.path:
        sys.path.append(_p)

import ml_dtypes  # noqa: E402

import concourse.bass as bass  # noqa: E402
import concourse.bass2jax as bass2jax  # noqa: E402
import concourse.mybir as mybir  # noqa: E402
import concourse.tile as tile  # noqa: E402
from concourse.bass_utils import (  # noqa: E402
    compile_bir_kernel as _orig_compile_bir_kernel,
    run_bass_kernel_spmd,
)

# The walrus build in this container accepts only ONE sync-wait per
# instruction ("Too many sync wait commands", CoreV3GenImpl setupSyncWait) —
# Tile's kernel-tail drain carries several. Split extra waits onto preceding
# single-wait EventSemaphore carriers on the same engine (engine program order
# makes the AND of waits equivalent).
import json as _json  # noqa: E402


def _split_multi_waits(bir_json):
    d = _json.loads(bir_json)
    changed = False
    for fn in d.get("functions", []):
        for bb in fn.get("blocks", []) or []:
            insts = bb.get("instructions")
            if not insts:
                continue
            out = []
            for ins in insts:
                si = ins.get("sync_info")
                waits = (si or {}).get("on_wait") or []
                if len(waits) > 1:
                    for j, w in enumerate(waits[:-1]):
                        out.append(
                            {
                                "engine": ins.get("engine"),
                                "ins": [],
                                "outs": [],
                                "name": f"{ins.get('name', 'I')}_w{j}",
                                "opcode": "EventSemaphore",
                                "sync_info": {"on_update": [], "on_wait": [w]},
                            }
                        )
                    si["on_wait"] = [waits[-1]]
                    changed = True
                out.append(ins)
            bb["instructions"] = out
    if not changed:
        return bir_json
    return _json.dumps(d).encode()


def _patched_compile_bir_kernel(bir_json, tmpdir, neff_name="file.neff"):
    return _orig_compile_bir_kernel(
        _split_multi_waits(bir_json), tmpdir, neff_name=neff_name
    )


bass2jax.compile_bir_kernel = _patched_compile_bir_kernel

P = 128
NCORES = 8
S, K_FULL, N_FULL = 4096, 4096, 16384
QMAX = 127.0
F32 = mybir.dt.float32
BF16 = mybir.dt.bfloat16
INT8 = mybir.dt.int8


def build_nc(S_C, K, N, NSUB=4, WK=8):
    """One-core program; SPMD-replicated across cores by the runner.

    Inputs (per core):
      qT  [KC, P, S_C] bf16 - host-quantized activations, transposed
      xsb [P, S_C] f32 - per-token scale, broadcast across partitions
      wt  [WC, NSUB, P, KC, P] bf16 - weights, host-packed SBUF-layout
      ws  [P, NT]   f32 - weight_scale packed ws[p, nt] = weight_scale[nt*128+p]
    Output:
      outT [N, S_C] f32 - dequantized output, transposed

    DMA plan: qT + first weight chunk are kc-chunked (WK blocks per piece)
    and interleaved so the first psum chain starts ~2 MB into the stream;
    the remaining weight chunks stream as whole (wc, sub) 1 MB slices on
    the gpsimd queue while qT/out traffic rides the sync queue.
    """
    KC = K // P  # contraction blocks
    NT = N // P  # output-channel tiles (one psum chain each)
    WC = NT // NSUB  # streamed weight chunks

    nc = bass.Bass()
    q8 = nc.declare_dram_parameter("q8", [P, KC, S_C], INT8, isOutput=False)
    xsb_d = nc.declare_dram_parameter("xsb", [P, S_C], F32, isOutput=False)
    wt = nc.declare_dram_parameter("wt", [WC, NSUB, P, KC, P], BF16, isOutput=False)
    ws = nc.declare_dram_parameter("ws", [P, NT], F32, isOutput=False)
    outT = nc.declare_dram_parameter("outT", [N, S_C], F32, isOutput=True)

    outT_t = outT.rearrange("(nt p) s -> nt p s", p=P)

    with tile.TileContext(nc) as tc, ExitStack() as ctx:
        const_pool = ctx.enter_context(tc.tile_pool(name="const", bufs=1))
        qpool = ctx.enter_context(tc.tile_pool(name="qp", bufs=1))
        wpool = ctx.enter_context(tc.tile_pool(name="wp", bufs=3))
        opool = ctx.enter_context(tc.tile_pool(name="op", bufs=4))
        ps_pool = ctx.enter_context(tc.tile_pool(name="psp", bufs=8, space="PSUM"))

        q8_sb = qpool.tile([P, KC, S_C], INT8)
        qT_sb = qpool.tile([P, KC, S_C], BF16)
        xsb = const_pool.tile([P, S_C], F32)
        ws_sb = const_pool.tile([P, NT], F32)

        # ---- PE pre-warm: junk matmuls on memset tiles ride out the DMA
        # wait and the tensor engine's cold-clock p-state ramp.
        junk_w = const_pool.tile([P, P], BF16)
        junk_r = const_pool.tile([P, S_C], BF16)
        nc.vector.memset(junk_w, 0.0)
        nc.vector.memset(junk_r, 0.0)
        junk_ps = ps_pool.tile([P, S_C], F32, name="ps", tag="ps")
        for i in range(14):
            nc.tensor.matmul(
                junk_ps, lhsT=junk_w, rhs=junk_r, start=(i == 0), stop=(i == 13)
            )
        junk_out = const_pool.tile([P, 1], F32)
        nc.vector.tensor_copy(junk_out, junk_ps[:, :1])

        # ---- DMA issue order (sync queue): q8 (kc-chunked) interleaved
        # with chunk-0 weights, then the small consts. Each q8 block is
        # upconverted int8->bf16 split across DVE/Act/Pool so conversion
        # keeps ahead of the first psum chain's consumption.
        def load_q(kc0, kcn):
            nc.sync.dma_start(
                q8_sb[:, kc0 : kc0 + kcn, :], q8[:, kc0 : kc0 + kcn, :]
            )

        def convert_q(kc0):
            # NOTE: keep this coarse (3 ops/block). Per-kc slicing measured
            # ~6 us WORSE on a fast-clock run: the fixed per-op engine
            # overhead of 32 tiny copies outweighs the conversion-latency
            # it saves, adding ~6 us of early chain stalls.
            nc.vector.tensor_copy(
                qT_sb[:, kc0 : kc0 + 3, :], q8_sb[:, kc0 : kc0 + 3, :]
            )
            nc.scalar.copy(
                qT_sb[:, kc0 + 3 : kc0 + 6, :], q8_sb[:, kc0 + 3 : kc0 + 6, :]
            )
            nc.gpsimd.tensor_copy(
                qT_sb[:, kc0 + 6 : kc0 + 8, :], q8_sb[:, kc0 + 6 : kc0 + 8, :]
            )

        def load_w_sub(wtile, wc, sub, engine, kc0=0, kcn=None):
            kcn = kcn if kcn is not None else KC
            engine.dma_start(
                wtile[:, sub, kc0 : kc0 + kcn, :], wt[wc, sub, :, kc0 : kc0 + kcn, :]
            )

        def load_wchunk(wc):
            wtile = wpool.tile([P, NSUB, KC, P], BF16, tag="wtile")
            for sub in range(NSUB):
                load_w_sub(wtile, wc, sub, nc.sync)
            return wtile

        wtiles = {}
        wtiles[0] = wpool.tile([P, NSUB, KC, P], BF16, name="wtile", tag="wtile")
        # Interleave the q8 stream with chunk-0 weights so chain 0 starts
        # ~1 MB into the stream and then drips along the q8 arrivals while
        # later chains' weights trickle in behind: per 8-block of kc, ship
        # q8[kc-block] then w0s0[kc-block] then w0s1[kc-block]; w0s2/s3 after.
        for kc0 in range(0, KC, 8):
            load_q(kc0, 8)
            load_w_sub(wtiles[0], 0, 0, nc.sync, kc0, 8)
            load_w_sub(wtiles[0], 0, 1, nc.sync, kc0, 8)
            convert_q(kc0)
        for sub in range(2, NSUB):
            load_w_sub(wtiles[0], 0, sub, nc.sync)
        nc.sync.dma_start(xsb, xsb_d[:, :])
        nc.sync.dma_start(ws_sb, ws[:, :])
        if WC > 1:
            wtiles[1] = load_wchunk(1)
        if WC > 2:
            wtiles[2] = load_wchunk(2)

        # ---- weights-stationary GEMM + fused dequant ----
        # 2-chunk-deep prefetch: issue wc+2's loads at iteration wc's top so
        # the transfer overlaps two full chunk-compute windows.
        for wc in range(WC):
            if wc + 2 < WC and (wc + 2) not in wtiles:
                wtiles[wc + 2] = load_wchunk(wc + 2)
            wtile = wtiles.pop(wc)
            if wc == 0:
                # Chains 0 and 1 interleaved kc-block-wise: one chain alone
                # consumes q8+w0 faster than the ~310 GB/s fabric delivers,
                # so alternating two chains over each delivered block turns
                # delivery-bound PE idle into back-to-back matmuls.
                pss = [
                    ps_pool.tile([P, S_C], F32, name="ps", tag="ps")
                    for _ in range(2)
                ]
                for kcb in range(0, KC, 8):
                    for sub in (0, 1):
                        for kc in range(kcb, kcb + 8):
                            nc.tensor.matmul(
                                pss[sub],
                                lhsT=wtile[:, sub, kc, :],
                                rhs=qT_sb[:, kc, :],
                                start=(kc == 0),
                                stop=(kc == KC - 1),
                            )
                sub_plan = [(0, pss[0]), (1, pss[1])] + [
                    (s, None) for s in range(2, NSUB)
                ]
            else:
                sub_plan = [(s, None) for s in range(NSUB)]
            for sub, ps in sub_plan:
                nt = wc * NSUB + sub
                if nt == NT - 1:
                    # final chain in column halves: the first half's eviction
                    # and out-DMA overlap the second half's matmuls, trimming
                    # the kernel tail ~0.4 us (start=True zeroes only the
                    # written columns on HW, so the half-groups are safe).
                    ps = ps_pool.tile([P, S_C], F32, name="ps", tag="ps")
                    h = S_C // 2
                    for c0 in (0, h):
                        for kc in range(KC):
                            nc.tensor.matmul(
                                ps[:, c0 : c0 + h],
                                lhsT=wtile[:, sub, kc, :],
                                rhs=qT_sb[:, kc, c0 : c0 + h],
                                start=(kc == 0),
                                stop=(kc == KC - 1),
                            )
                        out_sb = opool.tile([P, S_C], F32)
                        nc.vector.scalar_tensor_tensor(
                            out=out_sb[:, c0 : c0 + h],
                            in0=ps[:, c0 : c0 + h],
                            scalar=ws_sb[:, nt : nt + 1],
                            in1=xsb[:, c0 : c0 + h],
                            op0=mybir.AluOpType.mult,
                            op1=mybir.AluOpType.mult,
                        )
                        nc.sync.dma_start(
                            outT_t[nt][:, c0 : c0 + h], out_sb[:, c0 : c0 + h]
                        )
                    continue
                if ps is None:
                    ps = ps_pool.tile([P, S_C], F32, name="ps", tag="ps")
                    for kc in range(KC):
                        nc.tensor.matmul(
                            ps,
                            lhsT=wtile[:, sub, kc, :],
                            rhs=qT_sb[:, kc, :],
                            start=(kc == 0),
                            stop=(kc == KC - 1),
                        )
                out_sb = opool.tile([P, S_C], F32)
                # out = (acc * w_scale[n]) * x_scale[tok]
                nc.vector.scalar_tensor_tensor(
                    out=out_sb,
                    in0=ps,
                    scalar=ws_sb[:, nt : nt + 1],
                    in1=xsb,
                    op0=mybir.AluOpType.mult,
                    op1=mybir.AluOpType.mult,
                )
                nc.sync.dma_start(outT_t[nt], out_sb)

    return nc


def pack_inputs(input_tensor, weight, weight_scale, S_C, K, N, NSUB=4):
    """Host-side prep: quantize (bit-exact vs reference), transpose/shard q,
    pack weights to bf16 SBUF-chunk layout."""
    KC = K // P
    NT = N // P
    WC = NT // NSUB
    ncores = (input_tensor.size // K) // S_C

    x = np.ascontiguousarray(input_tensor.reshape(-1, K), dtype=np.float32)
    # per-token dynamic absmax int8 quantization (mirrors the f32 reference)
    amax = np.max(np.abs(x), axis=1, keepdims=True)
    x_scale = (np.maximum(amax, np.float32(1e-8)) / np.float32(QMAX)).astype(
        np.float32
    )
    q = np.clip(np.rint(x / x_scale), -QMAX, QMAX)
    # q8[c, p, kc, s] = q[c*S_C + s, kc*P + p]
    q8 = np.ascontiguousarray(
        q.astype(np.int8)
        .reshape(ncores, S_C, KC, P)
        .transpose(0, 3, 2, 1)
    )
    xsb = np.ascontiguousarray(
        np.broadcast_to(
            x_scale.reshape(ncores, 1, S_C), (ncores, P, S_C)
        )
    ).astype(np.float32)

    w_bf = weight.astype(ml_dtypes.bfloat16)  # [N, K], int8 values exact
    # wt[wc, sub, p, kc, i] = w[(wc*NSUB + sub)*P + i, kc*P + p]
    wt = np.ascontiguousarray(
        w_bf.reshape(WC, NSUB, P, KC, P).transpose(0, 1, 4, 3, 2)
    )
    ws = np.ascontiguousarray(
        weight_scale.reshape(NT, P).T.astype(np.float32)
    )  # [P, NT]
    return q8, xsb, wt, ws


@lru_cache(maxsize=2)
def _compiled_nc(S_C, K, N, NSUB, WK):
    return build_nc(S_C, K, N, NSUB=NSUB, WK=WK)


def run(input_tensor, weight, weight_scale, n_cores=NCORES, trace=False,
        NSUB=4, WK=8, **_compat_ignored):
    Sfull, K = input_tensor.shape[-2], input_tensor.shape[-1]
    N = weight.shape[0]
    S_C = Sfull // n_cores
    q8, xsb, wt, ws = pack_inputs(
        input_tensor, weight, weight_scale, S_C, K, N, NSUB
    )
    nc = _compiled_nc(S_C, K, N, NSUB, WK)
    in_maps = [
        {"q8": q8[c], "xsb": xsb[c], "wt": wt, "ws": ws}
        for c in range(n_cores)
    ]
    res = run_bass_kernel_spmd(nc, in_maps, core_ids=list(range(n_cores)), trace=trace)
    out = np.empty((Sfull, N), np.float32)
    for c in range(n_cores):
        out[c * S_C : (c + 1) * S_C] = res.results[c]["outT"].T
    return out[None], res


def kernel(input_tensor, weight, weight_scale):
    out, _ = run(
        np.asarray(input_tensor), np.asarray(weight), np.asarray(weight_scale)
    )
    return out
